# revision 26
# baseline (speedup 1.0000x reference)
"""TRN2 Bass kernel for nn_BrainModule (sparse_attention).

Computation (per sample b):
  emb[c,d]   = fourier embedding of positions[b,c]          (d = 242)
  scores[o,c]= heads[subj[b]][o,:] . emb[c,:] + offset[c]   (offset = -1e9 on
                                                             invalid channels)
  w[o,c]     = softmax_c(scores)
  out[o,t]   = sum_c w[o,c] * meg[b,c,t]

Strategy: data-parallel over batch B=32 across 8 cores (4 samples each).
On device, everything is computed in the [C, O] ("transposed") orientation so
the big einsum consumes the softmax weights directly as the matmul stationary
operand, with the 1/sum normalization folded into the PSUM->SBUF copy.
The score matmuls run in float32r (1 cycle/row at N>=256, measured HW
rel-err ~2e-4); the big einsum runs in bf16 (weights are softmax outputs in
[0,1], meg is unit-scale: measured end-to-end rel-err 3.3e-3) which halves
the meg DMA traffic and doubles the PE streaming rate.

The program is phase-ordered to keep the PE HAM-warm and to avoid ACT
table thrash:
  phase 1a: fourier embeddings, all samples batched into one [121, BS*C]
            DVE chain + 2 ACT Sins
  phase 1b: scores + exp + row-sums per sample (PE small MMs + ACT Exp)
  phase 2:  the big matmuls for all samples back-to-back (PE dense);
            meg loads ride the Sync (HWDGE) queue, output stores ride the
            GPSIMD (SWDGE) queue, so neither blocks the other's issue order

Channels past the valid prefix contribute exactly 0 weight (exp(-1e9) == 0
in fp32), so when the invalid channels form a suffix (always true for this
module: the last 16 channels are sentinel) the kernel is built for the
shorter channel prefix and skips their meg DMA entirely; otherwise it falls
back to all 273 channels with the -1e9 offset folded into the score matmul
as an extra K row.
"""
import numpy as np

B, C, T = 32, 273, 4096
CHOUT = 270
N_FREQS = 11
NF2 = N_FREQS * N_FREQS          # 121
D_A = NF2 + 1                    # cos half + offset/ones row
MARGIN = 0.2
WIDTH = 1.0 + 2.0 * MARGIN
INVALID = -0.1
NEG_INF = -1e9
N_CORES = 8
BS = B // N_CORES                # samples per core
CB = BS * C                      # batched embedding width
TWO_PI = float(2.0 * np.pi)
# largest f32 <= 2*pi, so |frac| = 0.5 never maps beyond pi
SCALE_2PI = float(np.nextafter(np.float32(2.0 * np.pi), np.float32(0.0)))

M_CHUNKS = [(0, 128), (128, 128), (256, CHOUT - 256)]  # partition chunks of O
TH = 2048                                              # meg/out tile t width
NT_Q = TH // 512                                       # 512-wide psum tiles

_NC_CACHE = {}


def _c_chunks(c_used):
    out = []
    c0 = 0
    while c0 < c_used:
        out.append((c0, min(128, c_used - c0)))
        c0 += 128
    return out


def _build_bass(c_used, robust_frac=False):
    import concourse.bacc as bacc
    import concourse.mybir as mybir
    import concourse.tile as tile
    import concourse.bass as bass

    F32 = mybir.dt.float32
    F32R = mybir.dt.float32r
    BF16 = mybir.dt.bfloat16
    I32 = mybir.dt.int32
    Sin = mybir.ActivationFunctionType.Sin
    Exp = mybir.ActivationFunctionType.Exp
    Copy = mybir.ActivationFunctionType.Copy
    F16 = mybir.dt.float16

    CC = _c_chunks(c_used)
    NCC = len(CC)
    # a trailing single-channel chunk is handled as one concurrent
    # row-tiled wave across the 4 t-chunks instead of 4 full 512-col passes
    K1_WAVE = CC[-1][1] == 1 and NT_Q == 4
    CCF = CC[:-1] if K1_WAVE else CC          # full chunks
    NF = len(CCF)

    nc = bacc.Bacc("TRN2", target_bir_lowering=False, debug=False,
                   num_devices=N_CORES)

    meg_d = nc.dram_tensor("meg", [BS, C, T], BF16, kind="ExternalInput")
    pa_d = nc.dram_tensor("pa", [BS, C], F32, kind="ExternalInput")
    pb_d = nc.dram_tensor("pb", [BS, C], F32, kind="ExternalInput")
    offs_d = nc.dram_tensor("offs", [BS, C], F32R, kind="ExternalInput")
    hta_d = nc.dram_tensor("hta", [BS, D_A, CHOUT], F32R, kind="ExternalInput")
    htb_d = nc.dram_tensor("htb", [BS, NF2, CHOUT], F32R, kind="ExternalInput")
    fi_d = nc.dram_tensor("fi", [NF2, 1], F32, kind="ExternalInput")
    fj_d = nc.dram_tensor("fj", [NF2, 1], F32, kind="ExternalInput")
    ones_d = nc.dram_tensor("ones", [128, 1], BF16, kind="ExternalInput")
    out_d = nc.dram_tensor("out", [BS, CHOUT, T], F16, kind="ExternalOutput")

    with tile.TileContext(nc) as tc:
        with (
            tc.tile_pool(name="const", bufs=1) as const,
            tc.tile_pool(name="emb1", bufs=2) as emb1,
            tc.tile_pool(name="wsb", bufs=4) as wsb,
            tc.tile_pool(name="persist", bufs=BS) as persist,
            tc.tile_pool(name="megp", bufs=3) as megp,
            tc.tile_pool(name="megp2", bufs=3) as megp2,
            tc.tile_pool(name="outp", bufs=3) as outp,
            tc.tile_pool(name="wps", bufs=1, space="PSUM") as wps,
            tc.tile_pool(name="bps", bufs=6, space="PSUM") as bps,
        ):
            megs_cache = {}

            NTH = T // TH

            def load_megs(b):
                if b in megs_cache:
                    return megs_cache.pop(b)
                megs = []
                for ci, (c0, cs) in enumerate(CCF):
                    pool = megp if cs > 64 else megp2
                    mg = pool.tile([cs, T], BF16, tag=f"mg{ci}")
                    nc.sync.dma_start(out=mg, in_=meg_d[b, c0:c0 + cs, :])
                    megs.append(mg)
                if K1_WAVE:
                    c0 = CC[-1][0]
                    # partition 32q holds t-chunk q of each t-half:
                    # [th0_q | th1_q] along the free dim
                    mgr = megp2.tile([97, NTH * 512], BF16, tag="mgr")
                    src = meg_d[b, c0, :].rearrange(
                        "(h q f) -> q h f", h=NTH, q=NT_Q)
                    dst = mgr[0:97:32, :].rearrange(
                        "q (h f) -> q h f", h=NTH)
                    nc.sync.dma_start(out=dst, in_=src)
                    megs.append(mgr)
                return megs

            def prefetch_megs(b):
                megs_cache[b] = load_megs(b)

            fi = const.tile([NF2, 1], F32, tag="fi")
            fj = const.tile([NF2, 1], F32, tag="fj")
            ones = const.tile([128, 1], BF16, tag="ones")
            nc.sync.dma_start(out=fi, in_=fi_d[:, :])
            nc.sync.dma_start(out=fj, in_=fj_d[:, :])
            nc.sync.dma_start(out=ones, in_=ones_d[:, :])

            # ---- phase 1a: fourier embeddings ---------------------------
            # emitted in two chunks (sample 0, then samples 1..3) so sample
            # 0's weight stage unblocks the PE as early as possible
            embAs = {}

            def emit_emb(b0, nb):
                w = nb * C
                a_rep = emb1.tile([NF2, w], F32, tag="s0")
                b_rep = emb1.tile([NF2, w], F32, tag="s1")
                pa_bcast = bass.AP(tensor=pa_d, offset=b0 * C,
                                   ap=[[0, NF2], [1, w]])
                pb_bcast = bass.AP(tensor=pb_d, offset=b0 * C,
                                   ap=[[0, NF2], [1, w]])
                nc.sync.dma_start(out=a_rep, in_=pa_bcast)
                nc.sync.dma_start(out=b_rep, in_=pb_bcast)

                xs = emb1.tile([NF2, w], F32, tag="s2")
                nc.vector.tensor_scalar_mul(out=xs, in0=a_rep, scalar1=fi)
                xs2 = emb1.tile([NF2, w], F32, tag="s3")
                nc.vector.tensor_scalar_mul(out=xs2, in0=b_rep, scalar1=fj)
                nc.vector.tensor_add(out=xs, in0=xs, in1=xs2)

                embA = emb1.tile([D_A, w], F32R, tag="embA")
                embB = emb1.tile([NF2, w], F32R, tag="embB")

                def reduce_frac(src):
                    ki = emb1.tile([NF2, w], I32, tag="ki")
                    kf = emb1.tile([NF2, w], F32, tag="kf")
                    frac = emb1.tile([NF2, w], F32, tag="fr")
                    # range reduction to [-0.5, 0.5] via f32->int32 cast.
                    # HW rounds to nearest so one stage suffices; CoreSim
                    # truncates, so sim builds add a comparison-based
                    # wraparound stage.
                    nc.vector.tensor_copy(ki, src)
                    nc.vector.tensor_copy(kf, ki)
                    nc.vector.tensor_sub(out=frac, in0=src, in1=kf)
                    if robust_frac:
                        nc.vector.tensor_scalar(
                            out=kf, in0=frac, scalar1=0.5, scalar2=None,
                            op0=mybir.AluOpType.is_gt)
                        nc.vector.tensor_sub(out=frac, in0=frac, in1=kf)
                        nc.vector.tensor_scalar(
                            out=kf, in0=frac, scalar1=-0.5, scalar2=None,
                            op0=mybir.AluOpType.is_lt)
                        nc.vector.tensor_add(out=frac, in0=frac, in1=kf)
                    return frac

                fr1 = reduce_frac(xs)
                nc.scalar.activation(out=embB, in_=fr1, func=Sin,
                                     scale=SCALE_2PI)
                # cos half: cos(2pi x) = sin(2pi (x + 0.25))
                nc.vector.tensor_scalar_add(out=xs2, in0=xs, scalar1=0.25)
                fr2 = reduce_frac(xs2)
                nc.scalar.activation(out=embA[0:NF2, :], in_=fr2, func=Sin,
                                     scale=SCALE_2PI)
                offs_flat = bass.AP(tensor=offs_d, offset=b0 * C,
                                    ap=[[w, 1], [1, w]])
                nc.sync.dma_start(out=embA[NF2:D_A, :], in_=offs_flat)
                for i in range(nb):
                    embAs[b0 + i] = (embA, embB, i * C)

            def emit_weight_stage(b):
                hta = wsb.tile([D_A, CHOUT], F32R, tag="hta")
                htb = wsb.tile([NF2, CHOUT], F32R, tag="htb")
                nc.sync.dma_start(out=hta, in_=hta_d[b, :, :])
                nc.sync.dma_start(out=htb, in_=htb_d[b, :, :])
                embA, embB, co = embAs[b]

                wt = []
                for ci, (c0, cs) in enumerate(CC):
                    ps_s = wps.tile([128, CHOUT], F32, tag="ps_s")
                    nc.tensor.matmul(ps_s[0:cs, :],
                                     embB[:, co + c0:co + c0 + cs], htb,
                                     start=True, stop=False)
                    nc.tensor.matmul(ps_s[0:cs, :],
                                     embA[:, co + c0:co + c0 + cs], hta,
                                     start=False, stop=True)
                    if K1_WAVE and ci == NCC - 1:
                        # single-channel chunk: replicate exp(weights) at
                        # partitions 0/32/64/96 for the row-tiled wave
                        wrep = persist.tile([97, CHOUT], BF16, tag="wrep")
                        for q in range(NT_Q):
                            nc.scalar.activation(
                                out=wrep[32 * q:32 * q + 1, :],
                                in_=ps_s[0:1, :], func=Exp)
                        wt.append(wrep)
                    else:
                        w_un = persist.tile([128, CHOUT], BF16,
                                            tag=f"w_un{ci}")
                        nc.scalar.activation(out=w_un[0:cs, :],
                                             in_=ps_s[0:cs, :], func=Exp)
                        wt.append(w_un)

                invs = []
                for mi, (m0, ms) in enumerate(M_CHUNKS):
                    ps_sum = wps.tile([128, 1], F32, tag="ps_sum")
                    for ci, (c0, cs) in enumerate(CC):
                        nc.tensor.matmul(ps_sum[0:ms, :],
                                         wt[ci][0:cs, m0:m0 + ms],
                                         ones[0:cs, :],
                                         start=(ci == 0), stop=(ci == NCC - 1))
                    inv = persist.tile([128, 1], F32, tag=f"inv{mi}")
                    nc.vector.reciprocal(out=inv[0:ms, :], in_=ps_sum[0:ms, :])
                    invs.append(inv)
                return wt, invs

            wts, invss = [None] * BS, [None] * BS
            emit_emb(0, 1)
            prefetch_megs(0)
            emit_emb(1, BS - 1)
            for b in range(BS):
                wts[b], invss[b] = emit_weight_stage(b)

            # ---- phase 2: big matmuls, PE back-to-back -----------------
            for b in range(BS):
                wt, invs = wts[b], invss[b]
                megs = load_megs(b)
                for th in range(T // TH):
                    t0 = th * TH
                    for mi, (m0, ms) in enumerate(M_CHUNKS):
                        ot = outp.tile([ms, TH], F16, tag=f"ot{mi}")
                        ps_list = []
                        for tq in range(NT_Q):
                            ps_o = bps.tile([128, 512], F32, tag="ps_o")
                            ps_list.append(ps_o)
                            for ci, (c0, cs) in enumerate(CCF):
                                nc.tensor.matmul(
                                    ps_o[0:ms, :],
                                    wt[ci][0:cs, m0:m0 + ms],
                                    megs[ci][:, t0 + tq * 512:
                                             t0 + (tq + 1) * 512],
                                    start=(ci == 0),
                                    stop=(not K1_WAVE and ci == NF - 1))
                        if K1_WAVE:
                            # single-channel contribution: 4 concurrent
                            # row-tiled K=1 matmuls (one per t-chunk)
                            for tq in range(NT_Q):
                                nc.tensor.matmul(
                                    ps_list[tq][0:ms, :],
                                    wt[-1][32 * tq:32 * tq + 1, m0:m0 + ms],
                                    megs[-1][32 * tq:32 * tq + 1,
                                             th * 512:(th + 1) * 512],
                                    start=False, stop=True,
                                    tile_position=(32 * tq, 0))
                        for tq in range(NT_Q):
                            # scaled psum->sbuf copy; alternate DVE/ACT so
                            # neither engine becomes the bottleneck
                            if tq % 2 == 0:
                                nc.vector.tensor_scalar_mul(
                                    out=ot[:, tq * 512:(tq + 1) * 512],
                                    in0=ps_list[tq][0:ms, :],
                                    scalar1=invs[mi][0:ms, :])
                            else:
                                nc.scalar.activation(
                                    out=ot[:, tq * 512:(tq + 1) * 512],
                                    in_=ps_list[tq][0:ms, :], func=Copy,
                                    scale=invs[mi][0:ms, :])
                        if b == BS - 1 and th == T // TH - 1:
                            nc.gpsimd.dma_start(
                                out=out_d[b, m0:m0 + ms, t0:t0 + TH // 2],
                                in_=ot[:, 0:TH // 2])
                            nc.gpsimd.dma_start(
                                out=out_d[b, m0:m0 + ms,
                                          t0 + TH // 2:t0 + TH],
                                in_=ot[:, TH // 2:TH])
                        else:
                            nc.gpsimd.dma_start(
                                out=out_d[b, m0:m0 + ms, t0:t0 + TH], in_=ot)

    nc.compile()
    return nc


def _get_nc(c_used):
    if c_used not in _NC_CACHE:
        _NC_CACHE[c_used] = _build_bass(c_used)
    return _NC_CACHE[c_used]


def _prep_host(meg, positions, subject_index, heads):
    """Build the 8 per-core input maps + pick the channel prefix length."""
    f32 = np.float32
    pos = np.asarray(positions, dtype=f32)
    a = ((pos[:, :, 0] + MARGIN) / WIDTH).astype(f32)           # [B, C]
    bcoord = ((pos[:, :, 1] + MARGIN) / WIDTH).astype(f32)      # [B, C]
    invalid = np.all(pos == INVALID, axis=-1)                   # [B, C]
    offs = np.where(invalid, f32(NEG_INF), f32(0.0)).astype(f32)

    # channels invalid in EVERY sample get weight exactly 0 (exp(-1e9)==0)
    # -> their meg data is never needed; use the valid prefix length
    valid_any = ~np.all(invalid, axis=0)                        # [C]
    c_used = int(np.max(np.nonzero(valid_any)[0])) + 1 if valid_any.any() else C

    h = np.asarray(heads, dtype=f32)[np.asarray(subject_index).astype(np.int64)]
    hT = np.ascontiguousarray(h.transpose(0, 2, 1))             # [B, 242, O]
    hta = np.concatenate(
        [hT[:, :NF2, :], np.ones((B, 1, CHOUT), dtype=f32)], axis=1)
    htb = np.ascontiguousarray(hT[:, NF2:, :])

    fr = np.arange(N_FREQS, dtype=f32)
    fi = np.repeat(fr, N_FREQS).reshape(NF2, 1)
    fj = np.tile(fr, N_FREQS).reshape(NF2, 1)
    import ml_dtypes as _mld
    ones = np.ones((128, 1), dtype=_mld.bfloat16)

    import ml_dtypes
    megf = np.asarray(meg, dtype=f32).astype(ml_dtypes.bfloat16)
    in_maps = []
    for c in range(N_CORES):
        s = slice(c * BS, (c + 1) * BS)
        in_maps.append(dict(
            meg=np.ascontiguousarray(megf[s]),
            pa=np.ascontiguousarray(a[s]),
            pb=np.ascontiguousarray(bcoord[s]),
            offs=np.ascontiguousarray(offs[s]),
            hta=np.ascontiguousarray(hta[s]),
            htb=np.ascontiguousarray(htb[s]),
            fi=fi, fj=fj, ones=ones,
        ))
    return in_maps, c_used


def kernel(meg, positions, subject_index, heads, _trace=False):
    from concourse.bass_utils import run_bass_kernel_spmd

    in_maps, c_used = _prep_host(meg, positions, subject_index, heads)
    nc = _get_nc(c_used)
    res = run_bass_kernel_spmd(nc, in_maps, core_ids=list(range(N_CORES)),
                               trace=_trace)
    out = np.concatenate([r["out"] for r in res.results], axis=0)
    if _trace:
        kernel.last_exec_time_ns = res.exec_time_ns
        kernel.last_results = res
    return out.astype(np.float32)


# revision 27
# speedup vs baseline: 1.1078x; 1.1078x over previous
"""TRN2 Bass kernel for nn_BrainModule (sparse_attention).

Computation (per sample b):
  emb[c,d]   = fourier embedding of positions[b,c]          (d = 242)
  scores[o,c]= heads[subj[b]][o,:] . emb[c,:] + offset[c]   (offset = -1e9 on
                                                             invalid channels)
  w[o,c]     = softmax_c(scores)
  out[o,t]   = sum_c w[o,c] * meg[b,c,t]

Strategy: data-parallel over batch B=32 across 8 cores (4 samples each).
On device, everything is computed in the [C, O] ("transposed") orientation so
the big einsum consumes the softmax weights directly as the matmul stationary
operand, with the 1/sum normalization folded into the PSUM->SBUF copy.
The score matmuls run in float32r (1 cycle/row at N>=256, measured HW
rel-err ~2e-4); the big einsum runs in bf16 (weights are softmax outputs in
[0,1], meg is unit-scale: measured end-to-end rel-err 3.3e-3) which halves
the meg DMA traffic and doubles the PE streaming rate.

The program is phase-ordered to keep the PE HAM-warm and to avoid ACT
table thrash:
  phase 1a: fourier embeddings, all samples batched into one [121, BS*C]
            DVE chain + 2 ACT Sins
  phase 1b: scores + exp + row-sums per sample (PE small MMs + ACT Exp)
  phase 2:  the big matmuls for all samples back-to-back (PE dense);
            meg loads ride the Sync (HWDGE) queue, output stores ride the
            GPSIMD (SWDGE) queue, so neither blocks the other's issue order

Channels past the valid prefix contribute exactly 0 weight (exp(-1e9) == 0
in fp32), so when the invalid channels form a suffix (always true for this
module: the last 16 channels are sentinel) the kernel is built for the
shorter channel prefix and skips their meg DMA entirely; otherwise it falls
back to all 273 channels with the -1e9 offset folded into the score matmul
as an extra K row.
"""
import numpy as np

B, C, T = 32, 273, 4096
CHOUT = 270
N_FREQS = 11
NF2 = N_FREQS * N_FREQS          # 121
D_A = NF2 + 1                    # cos half + offset/ones row
MARGIN = 0.2
WIDTH = 1.0 + 2.0 * MARGIN
INVALID = -0.1
NEG_INF = -1e9
N_CORES = 8
BS = B // N_CORES                # samples per core
CB = BS * C                      # batched embedding width
TWO_PI = float(2.0 * np.pi)
# largest f32 <= 2*pi, so |frac| = 0.5 never maps beyond pi
SCALE_2PI = float(np.nextafter(np.float32(2.0 * np.pi), np.float32(0.0)))

M_CHUNKS = [(0, 128), (128, 128), (256, CHOUT - 256)]  # partition chunks of O
TH = 2048                                              # meg/out tile t width
NT_Q = TH // 512                                       # 512-wide psum tiles

_NC_CACHE = {}


def _c_chunks(c_used):
    out = []
    c0 = 0
    while c0 < c_used:
        out.append((c0, min(128, c_used - c0)))
        c0 += 128
    return out


def _build_bass(c_used, robust_frac=False):
    import concourse.bacc as bacc
    import concourse.mybir as mybir
    import concourse.tile as tile
    import concourse.bass as bass

    F32 = mybir.dt.float32
    F32R = mybir.dt.float32r
    BF16 = mybir.dt.bfloat16
    I32 = mybir.dt.int32
    Sin = mybir.ActivationFunctionType.Sin
    Exp = mybir.ActivationFunctionType.Exp
    Copy = mybir.ActivationFunctionType.Copy
    F16 = mybir.dt.float16

    CC = _c_chunks(c_used)
    NCC = len(CC)
    # a trailing single-channel chunk is handled as one concurrent
    # row-tiled wave across the 4 t-chunks instead of 4 full 512-col passes
    K1_WAVE = CC[-1][1] == 1 and NT_Q == 4
    CCF = CC[:-1] if K1_WAVE else CC          # full chunks
    NF = len(CCF)

    nc = bacc.Bacc("TRN2", target_bir_lowering=False, debug=False,
                   num_devices=N_CORES)

    meg_d = nc.dram_tensor("meg", [BS, C, T], BF16, kind="ExternalInput")
    pa_d = nc.dram_tensor("pa", [BS, C], F32, kind="ExternalInput")
    pb_d = nc.dram_tensor("pb", [BS, C], F32, kind="ExternalInput")
    offs_d = nc.dram_tensor("offs", [BS, C], F32R, kind="ExternalInput")
    hta_d = nc.dram_tensor("hta", [BS, D_A, CHOUT], F32R, kind="ExternalInput")
    htb_d = nc.dram_tensor("htb", [BS, NF2, CHOUT], F32R, kind="ExternalInput")
    fi_d = nc.dram_tensor("fi", [NF2, 1], F32, kind="ExternalInput")
    fj_d = nc.dram_tensor("fj", [NF2, 1], F32, kind="ExternalInput")
    ones_d = nc.dram_tensor("ones", [128, 1], BF16, kind="ExternalInput")
    out_d = nc.dram_tensor("out", [BS, CHOUT, T], F16, kind="ExternalOutput")

    with tile.TileContext(nc) as tc:
        with (
            tc.tile_pool(name="const", bufs=1) as const,
            tc.tile_pool(name="emb1", bufs=2) as emb1,
            tc.tile_pool(name="wsb", bufs=4) as wsb,
            tc.tile_pool(name="persist", bufs=BS) as persist,
            tc.tile_pool(name="megp", bufs=3) as megp,
            tc.tile_pool(name="megp2", bufs=3) as megp2,
            tc.tile_pool(name="outp", bufs=3) as outp,
            tc.tile_pool(name="wps", bufs=1, space="PSUM") as wps,
            tc.tile_pool(name="bps", bufs=6, space="PSUM") as bps,
        ):
            megs_cache = {}

            NTH = T // TH

            def load_megs(b):
                if b in megs_cache:
                    return megs_cache.pop(b)
                megs = []
                for ci, (c0, cs) in enumerate(CCF):
                    pool = megp if cs > 64 else megp2
                    mg = pool.tile([cs, T], BF16, tag=f"mg{ci}")
                    nc.sync.dma_start(out=mg, in_=meg_d[b, c0:c0 + cs, :])
                    megs.append(mg)
                if K1_WAVE:
                    c0 = CC[-1][0]
                    # partition 32q holds t-chunk q of each t-half:
                    # [th0_q | th1_q] along the free dim
                    mgr = megp2.tile([97, NTH * 512], BF16, tag="mgr")
                    src = meg_d[b, c0, :].rearrange(
                        "(h q f) -> q h f", h=NTH, q=NT_Q)
                    dst = mgr[0:97:32, :].rearrange(
                        "q (h f) -> q h f", h=NTH)
                    nc.sync.dma_start(out=dst, in_=src)
                    megs.append(mgr)
                return megs

            def prefetch_megs(b):
                megs_cache[b] = load_megs(b)

            fi = const.tile([NF2, 1], F32, tag="fi")
            fj = const.tile([NF2, 1], F32, tag="fj")
            ones = const.tile([128, 1], BF16, tag="ones")
            nc.sync.dma_start(out=fi, in_=fi_d[:, :])
            nc.sync.dma_start(out=fj, in_=fj_d[:, :])
            nc.sync.dma_start(out=ones, in_=ones_d[:, :])

            # ---- phase 1a: fourier embeddings ---------------------------
            # emitted in two chunks (sample 0, then samples 1..3) so sample
            # 0's weight stage unblocks the PE as early as possible
            embAs = {}

            def emit_emb(b0, nb):
                w = nb * C
                a_rep = emb1.tile([NF2, w], F32, tag="s0")
                b_rep = emb1.tile([NF2, w], F32, tag="s1")
                pa_bcast = bass.AP(tensor=pa_d, offset=b0 * C,
                                   ap=[[0, NF2], [1, w]])
                pb_bcast = bass.AP(tensor=pb_d, offset=b0 * C,
                                   ap=[[0, NF2], [1, w]])
                nc.sync.dma_start(out=a_rep, in_=pa_bcast)
                nc.sync.dma_start(out=b_rep, in_=pb_bcast)

                xs = emb1.tile([NF2, w], F32, tag="s2")
                nc.vector.tensor_scalar_mul(out=xs, in0=a_rep, scalar1=fi)
                xs2 = emb1.tile([NF2, w], F32, tag="s3")
                nc.vector.tensor_scalar_mul(out=xs2, in0=b_rep, scalar1=fj)
                nc.vector.tensor_add(out=xs, in0=xs, in1=xs2)

                embA = emb1.tile([D_A, w], F32R, tag="embA")
                embB = emb1.tile([NF2, w], F32R, tag="embB")

                def reduce_frac(src):
                    ki = emb1.tile([NF2, w], I32, tag="ki")
                    kf = emb1.tile([NF2, w], F32, tag="kf")
                    frac = emb1.tile([NF2, w], F32, tag="fr")
                    # range reduction to [-0.5, 0.5] via f32->int32 cast.
                    # HW rounds to nearest so one stage suffices; CoreSim
                    # truncates, so sim builds add a comparison-based
                    # wraparound stage.
                    nc.vector.tensor_copy(ki, src)
                    nc.vector.tensor_copy(kf, ki)
                    nc.vector.tensor_sub(out=frac, in0=src, in1=kf)
                    if robust_frac:
                        nc.vector.tensor_scalar(
                            out=kf, in0=frac, scalar1=0.5, scalar2=None,
                            op0=mybir.AluOpType.is_gt)
                        nc.vector.tensor_sub(out=frac, in0=frac, in1=kf)
                        nc.vector.tensor_scalar(
                            out=kf, in0=frac, scalar1=-0.5, scalar2=None,
                            op0=mybir.AluOpType.is_lt)
                        nc.vector.tensor_add(out=frac, in0=frac, in1=kf)
                    return frac

                fr1 = reduce_frac(xs)
                nc.scalar.activation(out=embB, in_=fr1, func=Sin,
                                     scale=SCALE_2PI)
                # cos half: cos(2pi x) = sin(2pi (x + 0.25))
                nc.vector.tensor_scalar_add(out=xs2, in0=xs, scalar1=0.25)
                fr2 = reduce_frac(xs2)
                nc.scalar.activation(out=embA[0:NF2, :], in_=fr2, func=Sin,
                                     scale=SCALE_2PI)
                offs_flat = bass.AP(tensor=offs_d, offset=b0 * C,
                                    ap=[[w, 1], [1, w]])
                nc.sync.dma_start(out=embA[NF2:D_A, :], in_=offs_flat)
                for i in range(nb):
                    embAs[b0 + i] = (embA, embB, i * C)

            def emit_weight_stage(b):
                hta = wsb.tile([D_A, CHOUT], F32R, tag="hta")
                htb = wsb.tile([NF2, CHOUT], F32R, tag="htb")
                nc.sync.dma_start(out=hta, in_=hta_d[b, :, :])
                nc.sync.dma_start(out=htb, in_=htb_d[b, :, :])
                embA, embB, co = embAs[b]

                wt = []
                for ci, (c0, cs) in enumerate(CC):
                    ps_s = wps.tile([128, CHOUT], F32, tag="ps_s")
                    nc.tensor.matmul(ps_s[0:cs, :],
                                     embA[:, co + c0:co + c0 + cs], hta,
                                     start=True, stop=False)
                    nc.tensor.matmul(ps_s[0:cs, :],
                                     embB[:, co + c0:co + c0 + cs], htb,
                                     start=False, stop=True)
                    if K1_WAVE and ci == NCC - 1:
                        # single-channel chunk: replicate exp(weights) at
                        # partitions 0/32/64/96 for the row-tiled wave
                        wrep = persist.tile([97, CHOUT], BF16, tag="wrep")
                        for q in range(NT_Q):
                            nc.scalar.activation(
                                out=wrep[32 * q:32 * q + 1, :],
                                in_=ps_s[0:1, :], func=Exp)
                        wt.append(wrep)
                    else:
                        w_un = persist.tile([128, CHOUT], BF16,
                                            tag=f"w_un{ci}")
                        nc.scalar.activation(out=w_un[0:cs, :],
                                             in_=ps_s[0:cs, :], func=Exp)
                        wt.append(w_un)

                invs = []
                for mi, (m0, ms) in enumerate(M_CHUNKS):
                    ps_sum = wps.tile([128, 1], F32, tag="ps_sum")
                    for ci, (c0, cs) in enumerate(CC):
                        nc.tensor.matmul(ps_sum[0:ms, :],
                                         wt[ci][0:cs, m0:m0 + ms],
                                         ones[0:cs, :],
                                         start=(ci == 0), stop=(ci == NCC - 1))
                    inv = persist.tile([128, 1], F32, tag=f"inv{mi}")
                    nc.vector.reciprocal(out=inv[0:ms, :], in_=ps_sum[0:ms, :])
                    invs.append(inv)
                return wt, invs

            wts, invss = [None] * BS, [None] * BS
            emit_emb(0, 1)
            wts[0], invss[0] = emit_weight_stage(0)
            prefetch_megs(0)
            emit_emb(1, BS - 1)
            for b in range(1, BS):
                wts[b], invss[b] = emit_weight_stage(b)

            # ---- phase 2: big matmuls, PE back-to-back -----------------
            for b in range(BS):
                wt, invs = wts[b], invss[b]
                megs = load_megs(b)
                for th in range(T // TH):
                    t0 = th * TH
                    for mi, (m0, ms) in enumerate(M_CHUNKS):
                        ot = outp.tile([ms, TH], F16, tag=f"ot{mi}")
                        ps_list = []
                        for tq in range(NT_Q):
                            ps_o = bps.tile([128, 512], F32, tag="ps_o")
                            ps_list.append(ps_o)
                            for ci, (c0, cs) in enumerate(CCF):
                                nc.tensor.matmul(
                                    ps_o[0:ms, :],
                                    wt[ci][0:cs, m0:m0 + ms],
                                    megs[ci][:, t0 + tq * 512:
                                             t0 + (tq + 1) * 512],
                                    start=(ci == 0),
                                    stop=(not K1_WAVE and ci == NF - 1))
                        if K1_WAVE:
                            # single-channel contribution: 4 concurrent
                            # row-tiled K=1 matmuls (one per t-chunk)
                            for tq in range(NT_Q):
                                nc.tensor.matmul(
                                    ps_list[tq][0:ms, :],
                                    wt[-1][32 * tq:32 * tq + 1, m0:m0 + ms],
                                    megs[-1][32 * tq:32 * tq + 1,
                                             th * 512:(th + 1) * 512],
                                    start=False, stop=True,
                                    tile_position=(32 * tq, 0))
                        for tq in range(NT_Q):
                            # scaled psum->sbuf copy; alternate DVE/ACT so
                            # neither engine becomes the bottleneck
                            if tq % 2 == 0:
                                nc.vector.tensor_scalar_mul(
                                    out=ot[:, tq * 512:(tq + 1) * 512],
                                    in0=ps_list[tq][0:ms, :],
                                    scalar1=invs[mi][0:ms, :])
                            else:
                                nc.scalar.activation(
                                    out=ot[:, tq * 512:(tq + 1) * 512],
                                    in_=ps_list[tq][0:ms, :], func=Copy,
                                    scale=invs[mi][0:ms, :])
                        if b == BS - 1 and th == T // TH - 1:
                            nc.gpsimd.dma_start(
                                out=out_d[b, m0:m0 + ms, t0:t0 + TH // 2],
                                in_=ot[:, 0:TH // 2])
                            nc.gpsimd.dma_start(
                                out=out_d[b, m0:m0 + ms,
                                          t0 + TH // 2:t0 + TH],
                                in_=ot[:, TH // 2:TH])
                        else:
                            nc.gpsimd.dma_start(
                                out=out_d[b, m0:m0 + ms, t0:t0 + TH], in_=ot)

    nc.compile()
    return nc


def _get_nc(c_used):
    if c_used not in _NC_CACHE:
        _NC_CACHE[c_used] = _build_bass(c_used)
    return _NC_CACHE[c_used]


def _prep_host(meg, positions, subject_index, heads):
    """Build the 8 per-core input maps + pick the channel prefix length."""
    f32 = np.float32
    pos = np.asarray(positions, dtype=f32)
    a = ((pos[:, :, 0] + MARGIN) / WIDTH).astype(f32)           # [B, C]
    bcoord = ((pos[:, :, 1] + MARGIN) / WIDTH).astype(f32)      # [B, C]
    invalid = np.all(pos == INVALID, axis=-1)                   # [B, C]
    offs = np.where(invalid, f32(NEG_INF), f32(0.0)).astype(f32)

    # channels invalid in EVERY sample get weight exactly 0 (exp(-1e9)==0)
    # -> their meg data is never needed; use the valid prefix length
    valid_any = ~np.all(invalid, axis=0)                        # [C]
    c_used = int(np.max(np.nonzero(valid_any)[0])) + 1 if valid_any.any() else C

    h = np.asarray(heads, dtype=f32)[np.asarray(subject_index).astype(np.int64)]
    hT = np.ascontiguousarray(h.transpose(0, 2, 1))             # [B, 242, O]
    hta = np.concatenate(
        [hT[:, :NF2, :], np.ones((B, 1, CHOUT), dtype=f32)], axis=1)
    htb = np.ascontiguousarray(hT[:, NF2:, :])

    fr = np.arange(N_FREQS, dtype=f32)
    fi = np.repeat(fr, N_FREQS).reshape(NF2, 1)
    fj = np.tile(fr, N_FREQS).reshape(NF2, 1)
    import ml_dtypes as _mld
    ones = np.ones((128, 1), dtype=_mld.bfloat16)

    import ml_dtypes
    megf = np.asarray(meg, dtype=f32).astype(ml_dtypes.bfloat16)
    in_maps = []
    for c in range(N_CORES):
        s = slice(c * BS, (c + 1) * BS)
        in_maps.append(dict(
            meg=np.ascontiguousarray(megf[s]),
            pa=np.ascontiguousarray(a[s]),
            pb=np.ascontiguousarray(bcoord[s]),
            offs=np.ascontiguousarray(offs[s]),
            hta=np.ascontiguousarray(hta[s]),
            htb=np.ascontiguousarray(htb[s]),
            fi=fi, fj=fj, ones=ones,
        ))
    return in_maps, c_used


def kernel(meg, positions, subject_index, heads, _trace=False):
    from concourse.bass_utils import run_bass_kernel_spmd

    in_maps, c_used = _prep_host(meg, positions, subject_index, heads)
    nc = _get_nc(c_used)
    res = run_bass_kernel_spmd(nc, in_maps, core_ids=list(range(N_CORES)),
                               trace=_trace)
    out = np.concatenate([r["out"] for r in res.results], axis=0)
    if _trace:
        kernel.last_exec_time_ns = res.exec_time_ns
        kernel.last_results = res
    return out.astype(np.float32)
